# revision 9
# baseline (speedup 1.0000x reference)
"""Joint soft-histogram kernel for Trainium2 (Bass/Tile), 8-core data parallel.

Math (per batch b, K=256, L=1/256, W=L/2.5, N=65536 pixels):
    phi_k(x) = S_k(x) - S_{k+1}(x),   S_k(x) = sigmoid(640*x - 2.5*k)
    out[k, j] = sum_n phi_k(x_n) * phi_j(y_n) / N
             = (P[k,j] - P[k+1,j] - P[k,j+1] + P[k+1,j+1]) / N
    with P[k, j] = sum_n S_k(x_n) * S_j(y_n)   (double telescope).

P entries reach O(N) = 65536; a single fp32 PSUM accumulation would lose
~0.05 absolute to roundoff, which the double difference turns into ~5%
error.  So P is accumulated in SEGMENTS of 64 chunks (segment magnitude
<= 8192) in double-buffered PSUM regions, each segment flushed into an
fp32 SBUF accumulator by a DVE scalar_tensor_tensor add.  Total roundoff
~0.5% worst-entry, well under the 2e-2 tolerance.  Both adjacent
differences are applied once at the end: the j-diff on DVE, the k-diff
(across partitions) via three small matmuls against a constant bidiagonal
D^T matrix.

This leaves only 3 matmuls per chunk (k-rows 0:128, 128:256, 256) with no
per-chunk DVE diff or negation anywhere.

Per-chunk pre-adds (krow + 640*x_c) write fp16 from an fp16 krow tile:
tensor_scalar with a per-partition fp32 scalar AP runs in 4x DVE mode when
the tensor operands are 16-bit step-1 (measured ~287ns at FD=258,
dominated by the ~220-cycle instruction init).  fp16 argument rounding
only perturbs sigmoid args by <=2^-8 where non-saturated (~1e-3 rel-err
impact).  All staged pre-adds (x and y) share ONE group sigmoid
activation to amortize the ~290ns ScalarE per-instruction overhead.

NBIAS y-side chunks per group (placed FIRST so ScalarE computes them
while the DVE fills the staged pre-adds) skip the pre-add and use a
per-chunk activation with per-partition bias, balancing DVE vs ScalarE.

GPSIMD is not used at all: measured ~14.5ns/col for tensor ops AND it
stalls concurrent DVE ops via the shared SBUF port.

Sharding: pure data parallel, batch b -> core b.
"""

import numpy as np

import concourse.bass as bass
import concourse.tile as tile
from concourse import bacc, mybir
from concourse.bass_utils import run_bass_kernel_spmd

F32 = mybir.dt.float32
F16 = mybir.dt.float16

B = 8
K = 256
KB = 258              # sigmoid columns per chunk (j = 0..257; even for DVE modes)
NPIX = 65536
NCHUNK = 512
XG = 32               # chunks per staged group
NG = NCHUNK // XG     # 32 groups
NBIAS = 6             # per group: y-side chunks using bias-act (no pre-add)
SEG = 64              # chunks per PSUM accumulation segment
NSEG = NCHUNK // SEG
SCALE = 640.0
INV_N = 1.0 / NPIX

sig = mybir.ActivationFunctionType.Sigmoid
add = mybir.AluOpType.add
mult = mybir.AluOpType.mult

_cached_nc = None


def _build():
    nc = bacc.Bacc("TRN2")
    xd = nc.declare_dram_parameter("x", [128, 512], F32, isOutput=False)
    yd = nc.declare_dram_parameter("y", [128, 512], F32, isOutput=False)
    kd = nc.declare_dram_parameter("krow", [128, KB], F16, isOutput=False)
    # D^T pieces for the k-diff: dmat[r, k] = (k==r) - (k==r-1), r=0..256
    dd0 = nc.declare_dram_parameter("dmat0", [128, 256], F32, isOutput=False)
    dd1 = nc.declare_dram_parameter("dmat1", [128, 256], F32, isOutput=False)
    dd2 = nc.declare_dram_parameter("dmat2", [1, 256], F32, isOutput=False)
    od = nc.declare_dram_parameter("out", [256, 256], F32, isOutput=True)

    NSTAGE = XG - NBIAS   # y-side chunks staged via DVE pre-add
    GFX = XG * KB         # x-side staged free size (4128)
    GFY = NSTAGE * KB     # y-side staged free size
    GF = GFX + GFY        # merged stage width

    with tile.TileContext(nc) as tc:
        with (
            tc.tile_pool(name="singles", bufs=1) as singles,
            tc.tile_pool(name="stage", bufs=2) as stage,
            tc.tile_pool(name="work", bufs=2) as work,
            tc.tile_pool(name="psum", bufs=2, space="PSUM") as psum,
            tc.tile_pool(name="psum1", bufs=1, space="PSUM") as psum1,
        ):
            xt = singles.tile([128, 512], F32)
            nc.sync.dma_start(out=xt, in_=xd[:, :])
            yt = singles.tile([128, 512], F32)
            nc.sync.dma_start(out=yt, in_=yd[:, :])
            kr = singles.tile([128, KB], F16)
            nc.sync.dma_start(out=kr, in_=kd[:, :])
            dm0 = singles.tile([128, 256], F32)
            nc.sync.dma_start(out=dm0, in_=dd0[:, :])
            dm1 = singles.tile([128, 256], F32)
            nc.sync.dma_start(out=dm1, in_=dd1[:, :])
            dm2 = singles.tile([1, 256], F32)
            nc.sync.dma_start(out=dm2, in_=dd2[:, :])

            x6 = singles.tile([128, 512], F32)
            nc.vector.tensor_scalar_mul(out=x6, in0=xt, scalar1=SCALE)
            y6 = singles.tile([128, 512], F32)
            nc.vector.tensor_scalar_mul(out=y6, in0=yt, scalar1=SCALE)

            # fp32 SBUF accumulators for P (rows 0:128, 128:256, 256)
            accA = singles.tile([128, KB], F32)
            accB = singles.tile([128, KB], F32)
            accC = singles.tile([1, KB], F32)

            for s in range(NSEG):
                # double-buffered PSUM segment accumulators (pool bufs=2)
                PA = psum.tile([128, KB], F32, tag="PA")
                PB = psum.tile([128, KB], F32, tag="PB")
                # PC is padded to 128 partitions: an M=1 matmul (NumWeights=1)
                # cannot use fast-weight-load and serializes the LDW/MM
                # pipeline (measured 110 -> 203 ns/MM).  Rows 1..127 receive
                # garbage from the columns after o+256; only row 0 is read.
                PC = psum.tile([128, KB], F32, tag="PC")

                for gg in range(SEG // XG):
                    g = s * (SEG // XG) + gg
                    c0 = g * XG

                    # ---- bias-act y chunks (first NBIAS of the group)
                    syb = stage.tile([128, NBIAS * KB], F16, tag="syb")
                    for i in range(NBIAS):
                        nc.scalar.activation(
                            out=syb[:, i * KB : (i + 1) * KB],
                            in_=kr,
                            func=sig,
                            bias=y6[:, c0 + i : c0 + i + 1],
                            scale=1.0,
                        )

                    # ---- staged pre-adds -> ONE group sigmoid
                    axy = stage.tile([128, GF], F16, tag="axy")
                    for i in range(XG):
                        nc.vector.tensor_scalar(
                            out=axy[:, i * KB : (i + 1) * KB],
                            in0=kr,
                            scalar1=x6[:, c0 + i : c0 + i + 1],
                            scalar2=None,
                            op0=add,
                        )
                    for i in range(NBIAS, XG):
                        o = GFX + (i - NBIAS) * KB
                        nc.vector.tensor_scalar(
                            out=axy[:, o : o + KB],
                            in0=kr,
                            scalar1=y6[:, c0 + i : c0 + i + 1],
                            scalar2=None,
                            op0=add,
                        )
                    sxy = stage.tile([128, GF], F16, tag="sxy")
                    nc.scalar.activation(out=sxy, in_=axy, func=sig)

                    # ---- matmuls: 3 per chunk into the segment PSUM
                    for i in range(XG):
                        c = c0 + i
                        first = (c % SEG) == 0
                        last = (c % SEG) == SEG - 1
                        o = i * KB
                        if i < NBIAS:
                            rhs = syb[:, i * KB : i * KB + KB]
                        else:
                            oy = GFX + (i - NBIAS) * KB
                            rhs = sxy[:, oy : oy + KB]
                        nc.tensor.matmul(
                            PA, lhsT=sxy[:, o : o + 128], rhs=rhs,
                            start=first, stop=last,
                        )
                        nc.tensor.matmul(
                            PB, lhsT=sxy[:, o + 128 : o + 256], rhs=rhs,
                            start=first, stop=last,
                        )
                        nc.tensor.matmul(
                            PC, lhsT=sxy[:, o + 256 : o + 384], rhs=rhs,
                            start=first, stop=last,
                        )

                # ---- flush segment into fp32 SBUF accumulators
                for P, acc in ((PA, accA), (PB, accB), (PC[0:1, :], accC)):
                    if s == 0:
                        nc.vector.tensor_scalar_mul(out=acc, in0=P, scalar1=1.0)
                    else:
                        nc.vector.scalar_tensor_tensor(
                            out=acc, in0=P, scalar=1.0, in1=acc,
                            op0=mult, op1=add,
                        )

            # ---- endgame ----
            # j-diff on DVE: G[r, j] = P[r, j] - P[r, j+1], j = 0..256
            gA = work.tile([128, KB], F32, tag="gA")
            nc.vector.tensor_sub(out=gA[:, 0:257], in0=accA[:, 0:257],
                                 in1=accA[:, 1:258])
            gB = work.tile([128, KB], F32, tag="gB")
            nc.vector.tensor_sub(out=gB[:, 0:257], in0=accB[:, 0:257],
                                 in1=accB[:, 1:258])
            gC = work.tile([1, KB], F32, tag="gC")
            nc.vector.tensor_sub(out=gC[:, 0:257], in0=accC[:, 0:257],
                                 in1=accC[:, 1:258])
            # k-diff via D^T matmuls: out[k, j] = sum_r dmat[r, k] G[r, j]
            OUT = psum1.tile([128, 2, 256], F32)
            for h in range(2):
                nc.tensor.matmul(OUT[:, h, :], lhsT=dm0[:, 128 * h : 128 * h + 128],
                                 rhs=gA[:, 0:256], start=True, stop=False)
                nc.tensor.matmul(OUT[:, h, :], lhsT=dm1[:, 128 * h : 128 * h + 128],
                                 rhs=gB[:, 0:256], start=False, stop=False)
                nc.tensor.matmul(OUT[:, h, :], lhsT=dm2[:, 128 * h : 128 * h + 128],
                                 rhs=gC[:, 0:256], start=False, stop=True)
            for h in range(2):
                t1 = work.tile([128, 256], F32, tag="ep")
                nc.scalar.activation(
                    out=t1, in_=OUT[:, h, :],
                    func=mybir.ActivationFunctionType.Copy, scale=INV_N,
                )
                nc.sync.dma_start(out=od[128 * h : 128 * (h + 1), :], in_=t1)

    nc.finalize()
    return nc


def _get_nc():
    global _cached_nc
    if _cached_nc is None:
        _cached_nc = _build()
    return _cached_nc


def _krow():
    row = (np.arange(KB, dtype=np.float32) * np.float32(-2.5)).astype(np.float16)
    return np.tile(row[None, :], (128, 1))


def _dmats():
    # dmat[r, k] = (k==r) - (k==r-1), r = 0..256, k = 0..255
    dm = np.zeros((257, 256), np.float32)
    for r in range(257):
        if r < 256:
            dm[r, r] = 1.0
        if r >= 1:
            dm[r, r - 1] = -1.0
    return dm[0:128], dm[128:256], dm[256:257]


def _in_maps(x, y):
    x = np.ascontiguousarray(np.asarray(x, dtype=np.float32))
    y = np.ascontiguousarray(np.asarray(y, dtype=np.float32))
    kr = _krow()
    d0, d1, d2 = _dmats()
    return [
        {
            "x": x[b].reshape(128, 512),
            "y": y[b].reshape(128, 512),
            "krow": kr,
            "dmat0": d0,
            "dmat1": d1,
            "dmat2": d2,
        }
        for b in range(B)
    ]


def run(x, y, trace=False, **trace_kw):
    """Run on all 8 cores; returns (out (8,256,256) f32, BassKernelResults)."""
    nc = _get_nc()
    res = run_bass_kernel_spmd(nc, _in_maps(x, y), list(range(B)), trace=trace,
                               **trace_kw)
    out = np.stack([res.results[b]["out"] for b in range(B)]).astype(np.float32)
    return out, res


def kernel(x, y):
    out, _ = run(x, y)
    return out
